# revision 1
# baseline (speedup 1.0000x reference)
"""Softmax-gated GRU on 8 trn2 NeuronCores — LDW-grouped, coarse-DVE phase 2.

Data-parallel over batch (bs=128 -> 16/core, weights replicated).
Phase 2 per step, ordered to minimize PE stationary reloads (k-outer) and
DVE instruction count (coarse [16,1024] ops, bf16 where increments are small):
  zr:  4 X-injects share one eye16 load; then k-outer: one hT_k load feeds
       all 4 zr slices.  r slices first so exp_r/sums overlap the z half.
  u:   er*h and *(1/S_r) as two [16,1024] ops (bf16 out), 8 consecutive
       transposes share the identity stationary.
  cand: 2 X-injects grouped; k-outer over uT chunks.
  h':  per 512-slice: tanh, (th-h), *ez, *(1/S_z), h+=, then that slice's 4
       transposes — slice 0's tail overlaps slice 1's candidate matmuls.
"""

import sys

sys.path.insert(0, "/opt/trn_rl_repo")

import contextlib
import os

import numpy as np
import ml_dtypes

import concourse.bass as bass
import concourse.bacc as bacc_mod
import concourse.tile as tile
from concourse import mybir
from concourse.bass_utils import run_bass_kernel_spmd

SEQ, BS, IN, H = 512, 128, 512, 1024
NCORES = 8
BSC = BS // NCORES          # 16 batch rows per core
KCH = H // 128              # 8 contraction chunks (h part)
KXC = IN // 128             # 4 contraction chunks (x part)
NTOT = 3 * H                # gates [z, r, h] concatenated: 3072
NSL = 512                 # phase-1 slice
NSL2 = 512                # phase-2 slice
F32, BF16 = mybir.dt.float32, mybir.dt.bfloat16
AF = mybir.ActivationFunctionType

TANH_TABLE = True


def build_nc(seq=SEQ):
    nc = bacc_mod.Bacc()
    xT = nc.declare_dram_parameter("xT", [KXC, 128, seq, BSC], BF16, isOutput=False)
    h0 = nc.declare_dram_parameter("h0", [BSC, H], F32, isOutput=False)
    whh = nc.declare_dram_parameter("whh", [KCH, 128, NTOT], BF16, isOutput=False)
    wxx = nc.declare_dram_parameter("wxx", [KXC, 128, NTOT], BF16, isOutput=False)
    bias = nc.declare_dram_parameter("bias", [1, NTOT], BF16, isOutput=False)
    ident = nc.declare_dram_parameter("ident", [128, 128], F32, isOutput=False)
    eye16 = nc.declare_dram_parameter("eye16", [BSC, BSC], BF16, isOutput=False)
    ones1 = nc.declare_dram_parameter("ones1", [1, 128], BF16, isOutput=False)
    out = nc.declare_dram_parameter("out", [seq, BSC, H], F32, isOutput=True)
    xdram = nc.dram_tensor("xscratch", [seq, BSC, NTOT], BF16)

    n_mt = seq * BSC // 128  # phase-1 M-tiles (8 steps each)

    with tile.TileContext(nc) as tc:
        with contextlib.ExitStack() as ctx:
            consts = ctx.enter_context(tc.tile_pool(name="consts", bufs=1))
            wpool = ctx.enter_context(tc.tile_pool(name="w", bufs=1))
            xtp = ctx.enter_context(tc.tile_pool(name="xt", bufs=3))
            stg = ctx.enter_context(tc.tile_pool(name="stg", bufs=3))
            xs = ctx.enter_context(tc.tile_pool(name="xs", bufs=4))
            hpool = ctx.enter_context(tc.tile_pool(name="h", bufs=3))
            tails = ctx.enter_context(tc.tile_pool(name="tails", bufs=3))
            scal = ctx.enter_context(tc.tile_pool(name="scal", bufs=3))
            ps_r = ctx.enter_context(tc.tile_pool(name="ps_r", bufs=1, space="PSUM"))
            ps_z = ctx.enter_context(tc.tile_pool(name="ps_z", bufs=1, space="PSUM"))
            ps_h = ctx.enter_context(tc.tile_pool(name="ps_h", bufs=1, space="PSUM"))
            ps_tr = ctx.enter_context(tc.tile_pool(name="ps_tr", bufs=1, space="PSUM"))
            ps_x = ctx.enter_context(tc.tile_pool(name="ps_x", bufs=1, space="PSUM"))

            w_sb = wpool.tile([128, KCH, NTOT], BF16)
            nc.sync.dma_start(w_sb[:], whh.rearrange("k p n -> p k n"))
            wx_sb = wpool.tile([128, KXC, NTOT], BF16)
            nc.sync.dma_start(wx_sb[:], wxx.rearrange("k p n -> p k n"))
            b_sb = consts.tile([1, NTOT], BF16)
            nc.sync.dma_start(b_sb[:], bias[:])
            id_sb = consts.tile([128, 128], F32)
            nc.sync.dma_start(id_sb[:], ident[:])
            e16_sb = consts.tile([BSC, BSC], BF16)
            nc.sync.dma_start(e16_sb[:], eye16[:])
            on_sb = consts.tile([1, 128], BF16)
            nc.sync.dma_start(on_sb[:], ones1[:])

            # ------- phase 1 (lazy): X = x @ W_x + b -> DRAM, one m-tile -------
            NPS = NTOT // NSL  # 6 slices

            def emit_mtile(mt):
                xt_sb = xtp.tile([128, KXC, 128], BF16, tag="xt")
                nc.sync.dma_start(
                    xt_sb[:],
                    xT[:, :, mt * 8 : (mt + 1) * 8, :]
                    .rearrange("k p t b -> p k (t b)"),
                )
                stage = stg.tile([128, NTOT], BF16, tag="stage")
                for s in range(NPS):
                    px = ps_x.tile([128, NSL], F32, tag="px")
                    nc.tensor.matmul(
                        px[:], on_sb[:], b_sb[:, s * NSL : (s + 1) * NSL],
                        start=True, stop=False,
                    )
                    for k in range(KXC):
                        nc.tensor.matmul(
                            px[:], xt_sb[:, k, :],
                            wx_sb[:, k, s * NSL : (s + 1) * NSL],
                            start=False, stop=(k == KXC - 1),
                        )
                    if s % 2:
                        nc.scalar.copy(stage[:, s * NSL : (s + 1) * NSL], px[:])
                    else:
                        nc.vector.tensor_copy(stage[:, s * NSL : (s + 1) * NSL], px[:])
                nc.sync.dma_start(
                    xdram[mt * 8 : (mt + 1) * 8].rearrange("t b n -> (t b) n"),
                    stage[:],
                )

            P1_LEAD = 5  # m-tiles emitted ahead of the consuming step
            for mt in range(min(P1_LEAD, n_mt)):
                emit_mtile(mt)

            p1 = {"mt": P1_LEAD, "s": 0, "xt": None, "stage": None}

            def emit_slice():
                if p1["mt"] >= n_mt:
                    return
                mt, s = p1["mt"], p1["s"]
                if s == 0:
                    p1["xt"] = xtp.tile([128, KXC, 128], BF16, tag="xt", name="p1xt")
                    nc.sync.dma_start(
                        p1["xt"][:],
                        xT[:, :, mt * 8 : (mt + 1) * 8, :]
                        .rearrange("k p t b -> p k (t b)"),
                    )
                    p1["stage"] = stg.tile([128, NTOT], BF16, tag="stage", name="p1stage")
                xt_sb, stage = p1["xt"], p1["stage"]
                px = ps_x.tile([128, NSL], F32, tag="px")
                nc.tensor.matmul(
                    px[:], on_sb[:], b_sb[:, s * NSL : (s + 1) * NSL],
                    start=True, stop=False,
                )
                for k in range(KXC):
                    nc.tensor.matmul(
                        px[:], xt_sb[:, k, :],
                        wx_sb[:, k, s * NSL : (s + 1) * NSL],
                        start=False, stop=(k == KXC - 1),
                    )
                if s % 2:
                    nc.scalar.copy(stage[:, s * NSL : (s + 1) * NSL], px[:])
                else:
                    nc.vector.tensor_copy(stage[:, s * NSL : (s + 1) * NSL], px[:])
                if s == NPS - 1:
                    nc.sync.dma_start(
                        xdram[mt * 8 : (mt + 1) * 8].rearrange("t b n -> (t b) n"),
                        stage[:],
                    )
                    p1["mt"], p1["s"] = mt + 1, 0
                else:
                    p1["s"] = s + 1

            # ---------------- phase 2: recurrence ----------------
            h_bm = hpool.tile([BSC, H], F32, tag="h_bm")
            nc.sync.dma_start(h_bm[:], h0[:])
            trp = ps_x.tile([128, 128], F32, tag="px")
            for c in range(KCH):
                nc.tensor.transpose(
                    trp[:, c * BSC : (c + 1) * BSC],
                    h_bm[:, c * 128 : (c + 1) * 128],
                    id_sb[:BSC, :BSC],
                )
            hT = hpool.tile([128, 128], BF16, tag="hT")
            nc.vector.tensor_copy(hT[:], trp[:])
            hTf = hpool.tile([128, 128], F32, tag="hTf")
            nc.vector.tensor_copy(hTf[:], trp[:])
            h_bf = hpool.tile([BSC, H], BF16, tag="h_bf")
            nc.vector.tensor_copy(h_bf[:], h_bm[:])
            hTf = hpool.tile([128, 128], F32, tag="hTf")
            nc.vector.tensor_copy(hTf[:], trp[:])

            ZRS = (H, H + NSL2, 0, NSL2)  # r slices first, then z
            for t in range(seq):
                xx = xs.tile([BSC, NTOT], BF16, tag="xx")
                nc.sync.dma_start(xx[:], xdram[t])

                # z,r: injects grouped (one eye16 load); r k-loop first so
                # its exp/u tail overlaps the z k-loop on ACT/DVE.
                pr = ps_r.tile([BSC, H], F32, tag="pr")
                pz = ps_z.tile([BSC, H], F32, tag="pz")
                psel = {0: pz, NSL2: pz, H: pr, H + NSL2: pr}
                for lo in ZRS:
                    nc.tensor.matmul(
                        psel[lo][:, lo % H : lo % H + NSL2],
                        e16_sb[:], xx[:, lo : lo + NSL2],
                        start=True, stop=False,
                    )
                for k in range(KCH):
                    for lo in ZRS[: len(ZRS) // 2]:
                        nc.tensor.matmul(
                            psel[lo][:, lo % H : lo % H + NSL2],
                            hT[:, k * BSC : (k + 1) * BSC],
                            w_sb[:, k, lo : lo + NSL2],
                            start=False, stop=(k == KCH - 1),
                        )

                # r softmax pieces; u = (e_r/S_r) o h  (coarse ops)
                er = tails.tile([BSC, H], F32, tag="er")
                s_r = scal.tile([BSC, 1], F32, tag="s_r")
                nc.scalar.activation(er[:], pr[:], AF.Exp, accum_out=s_r[:])
                nc.vector.reciprocal(s_r[:], s_r[:])
                u_c = tails.tile([BSC, H], BF16, tag="u_c")
                nc.vector.tensor_mul(u_c[:], er[:], h_bm[:])
                nc.vector.tensor_scalar_mul(u_c[:], u_c[:], s_r[:])

                for k in range(KCH):
                    for lo in ZRS[len(ZRS) // 2 :]:
                        nc.tensor.matmul(
                            psel[lo][:, lo % H : lo % H + NSL2],
                            hT[:, k * BSC : (k + 1) * BSC],
                            w_sb[:, k, lo : lo + NSL2],
                            start=False, stop=(k == KCH - 1),
                        )
                if t % 2 == 1:
                    emit_slice()
                trp_u = ps_tr.tile([128, 128], BF16, tag="trb")
                for c in range(KCH):
                    nc.tensor.transpose(
                        trp_u[:, c * BSC : (c + 1) * BSC],
                        u_c[:, c * 128 : (c + 1) * 128],
                        e16_sb[:],
                    )
                uT = hpool.tile([128, 128], BF16, tag="uT")
                nc.vector.tensor_copy(uT[:], trp_u[:])

                # candidate: injects grouped, then k-outer
                ph = ps_h.tile([BSC, H], F32, tag="ph")
                for s in range(H // NSL2):
                    lo = 2 * H + s * NSL2
                    nc.tensor.matmul(
                        ph[:, s * NSL2 : (s + 1) * NSL2],
                        e16_sb[:], xx[:, lo : lo + NSL2],
                        start=True, stop=False,
                    )
                for k in range(KCH):
                    for s in range(H // NSL2):
                        nc.tensor.matmul(
                            ph[:, s * NSL2 : (s + 1) * NSL2],
                            uT[:, k * BSC : (k + 1) * BSC],
                            w_sb[:, k, 2 * H + s * NSL2 : 2 * H + (s + 1) * NSL2],
                            start=False, stop=(k == KCH - 1),
                        )

                if t % 2 == 0:
                    emit_slice()

                # z softmax (overlaps candidate matmuls)
                ez = tails.tile([BSC, H], BF16, tag="ez")
                s_z = scal.tile([BSC, 1], F32, tag="s_z")
                nc.scalar.activation(ez[:], pz[:], AF.Exp, accum_out=s_z[:])
                nc.vector.reciprocal(s_z[:], s_z[:])
                nc.vector.tensor_scalar_mul(ez[:], ez[:], s_z[:])

                # h' per 512-slice: 2-op prefix (sub, mul with pre-scaled ez),
                # transpose the bf16 increment, accumulate transposed f32 h.
                h_new = hpool.tile([BSC, H], F32, tag="h_bm")
                trp_h = ps_tr.tile([128, 128], BF16, tag="trb")
                hT_new = hpool.tile([128, 128], BF16, tag="hT")
                hTf_new = hpool.tile([128, 128], F32, tag="hTf")
                th = tails.tile([BSC, H], BF16, tag="th")
                mz = tails.tile([BSC, H], BF16, tag="mz")
                for s in range(H // NSL):
                    sl = slice(s * NSL, (s + 1) * NSL)
                    hsl = slice(s * 64, (s + 1) * 64)
                    nc.scalar.activation(th[:, sl], ph[:, sl], AF.Tanh)
                    nc.vector.tensor_sub(mz[:, sl], th[:, sl], h_bf[:, sl])
                    nc.vector.tensor_mul(mz[:, sl], ez[:, sl], mz[:, sl])
                    if t + 1 < seq:
                        for c in range(s * KCH // 2, (s + 1) * KCH // 2):
                            nc.tensor.transpose(
                                trp_h[:, c * BSC : (c + 1) * BSC],
                                mz[:, c * 128 : (c + 1) * 128],
                                e16_sb[:],
                            )
                        nc.vector.tensor_add(
                            hTf_new[:, hsl], hTf[:, hsl], trp_h[:, hsl]
                        )
                        nc.vector.tensor_copy(hT_new[:, hsl], hTf_new[:, hsl])
                    nc.vector.tensor_add(h_new[:, sl], h_bm[:, sl], mz[:, sl])
                nc.sync.dma_start(out[t], h_new[:])
                h_bf_new = hpool.tile([BSC, H], BF16, tag="h_bf")
                nc.vector.tensor_copy(h_bf_new[:], h_new[:])
                h_bm = h_new
                hT = hT_new
                hTf = hTf_new
                h_bf = h_bf_new
    nc.compile()
    return nc


def prep_inputs(x, h0, Wz, bz, Wr, br, Wh, bh, seq=SEQ):
    bf = ml_dtypes.bfloat16
    whh = np.concatenate([Wz[:H], Wr[:H], Wh[:H]], axis=1)   # [1024, 3072]
    wxx = np.concatenate([Wz[H:], Wr[H:], Wh[H:]], axis=1)   # [512, 3072]
    bias = np.concatenate([bz, br, bh])[None, :]
    shared = {
        "whh": np.ascontiguousarray(whh.reshape(KCH, 128, NTOT)).astype(bf),
        "wxx": np.ascontiguousarray(wxx.reshape(KXC, 128, NTOT)).astype(bf),
        "bias": bias.astype(bf),
        "ident": np.eye(128, dtype=np.float32),
        "eye16": np.eye(BSC, dtype=bf),
        "ones1": np.ones((1, 128), dtype=bf),
    }
    maps = []
    for c in range(NCORES):
        sl = slice(c * BSC, (c + 1) * BSC)
        xc = np.asarray(x[:seq, sl, :], dtype=np.float32)
        xTc = np.ascontiguousarray(xc.transpose(2, 0, 1)).reshape(
            KXC, 128, seq, BSC
        )
        maps.append(dict(shared, xT=xTc.astype(bf), h0=np.asarray(h0[sl], np.float32)))
    return maps


LAST_EXEC_NS = None


def kernel(x, h0, Wz, bz, Wr, br, Wh, bh):
    global LAST_EXEC_NS
    nc = build_nc(SEQ)
    maps = prep_inputs(x, h0, Wz, bz, Wr, br, Wh, bh, SEQ)
    res = run_bass_kernel_spmd(nc, maps, list(range(NCORES)))
    if res.exec_time_ns is not None:
        LAST_EXEC_NS = res.exec_time_ns
    outs = [np.asarray(res.results[c]["out"]) for c in range(NCORES)]
    return np.concatenate(outs, axis=1).astype(np.float32)

